# revision 3
# baseline (speedup 1.0000x reference)
"""Cross-attention Trainium2 kernel, tensor-parallel over heads across 8 NeuronCores.

Problem (hardcoded): B=2, S1=S2=1024 (concat -> S=2048), E=1024, H=16, D=64.
Sharding: 2 heads per core (Megatron-style TP). Each core computes
  qT/kT/vT = column-sharded projections of y = concat(x1, x2)      [128, 4096]
  attnT    = softmax(q k^T / sqrt(D)) v   for its 2 heads          [64, 2, 4096]
  outT_c   = Wo[:, shard]^T-partial output projection              [1024, 4096]
Host sums the 8 partial outT's, transposes, adds bo.

All matmuls in bf16 (fp32 PSUM accumulate); softmax in fp32 on the ACT engine.
"""

import numpy as np
import ml_dtypes

BF16 = ml_dtypes.bfloat16

# Problem dims (hardcoded per contract)
B = 2
S = 2048          # concat sequence length per batch
E = 1024
H = 16
D = 64
NCORES = 8
HPC = H // NCORES  # heads per core = 2
EPC = HPC * D      # feature shard per core = 128
R = B * S          # flattened (b, s) columns = 4096
KB = E // 128      # contraction blocks for projections = 8


def build_program(seq=S, nbatch=B):
    """Build the SPMD Bass program (identical on all cores)."""
    from contextlib import ExitStack

    import concourse.bacc as bacc
    import concourse.tile as tile
    from concourse import mybir
    from concourse.masks import make_identity

    bf = mybir.dt.bfloat16
    f32 = mybir.dt.float32
    AF = mybir.ActivationFunctionType

    r = nbatch * seq            # total columns
    ntt = seq // 128            # t-tiles per batch
    sh = 1024 if seq % 1024 == 0 else seq   # s-half width
    nsh = seq // sh             # s-halves per batch
    assert seq % 128 == 0 and sh % 512 == 0 or sh < 512

    def nsplit(width, maxw=512):
        # split a free-dim width into <=512 chunks (PSUM bank limit per matmul)
        out = []
        o = 0
        while o < width:
            w = min(maxw, width - o)
            out.append((o, w))
            o += w
        return out

    nc = bacc.Bacc(
        "TRN2",
        target_bir_lowering=False,
        debug=False,
        enable_asserts=False,
        num_devices=NCORES,
    )

    # DRAM parameters (per-core data differs, program identical)
    yt_d = nc.dram_tensor("yt", [KB, 128, r], bf, kind="ExternalInput").ap()
    wq_d = nc.dram_tensor("wq", [128, KB, 128], bf, kind="ExternalInput").ap()
    wk_d = nc.dram_tensor("wk", [128, KB, 128], bf, kind="ExternalInput").ap()
    wv_d = nc.dram_tensor("wv", [128, KB, 128], bf, kind="ExternalInput").ap()
    wo_d = nc.dram_tensor("wo", [64, HPC, KB, 128], bf, kind="ExternalInput").ap()
    bq_d = nc.dram_tensor("bq", [128, 1], f32, kind="ExternalInput").ap()
    bk_d = nc.dram_tensor("bk", [128, 1], f32, kind="ExternalInput").ap()
    bv_d = nc.dram_tensor("bv", [128, 1], f32, kind="ExternalInput").ap()
    out_d = nc.dram_tensor("outp", [E, r], f32, kind="ExternalOutput").ap()

    with tile.TileContext(nc) as tc, ExitStack() as ctx:
        consts = ctx.enter_context(tc.tile_pool(name="consts", bufs=1))
        big = ctx.enter_context(tc.tile_pool(name="big", bufs=1))
        wt_pool = ctx.enter_context(tc.tile_pool(name="wtp", bufs=3))
        rec_pool = ctx.enter_context(tc.tile_pool(name="recp", bufs=2))

        # constants / weights
        wq_sb = consts.tile([128, KB, 128], bf)
        wk_sb = consts.tile([128, KB, 128], bf)
        wv_sb = consts.tile([128, KB, 128], bf)
        wo_sb = consts.tile([64, HPC, KB, 128], bf)
        bq_sb = consts.tile([128, 1], f32)
        bk_sb = consts.tile([128, 1], f32)
        bv_sb = consts.tile([128, 1], f32)
        ident = consts.tile([128, 128], bf)
        ones_sb = consts.tile([65, 64], bf)
        yt_sb = consts.tile([128, KB, r], bf)

        nc.sync.dma_start(out=wq_sb, in_=wq_d)
        nc.sync.dma_start(out=wk_sb, in_=wk_d)
        nc.sync.dma_start(out=wv_sb, in_=wv_d)
        nc.sync.dma_start(out=wo_sb, in_=wo_d)
        nc.sync.dma_start(out=bq_sb, in_=bq_d)
        nc.sync.dma_start(out=bk_sb, in_=bk_d)
        nc.sync.dma_start(out=bv_sb, in_=bv_d)
        for kb in range(KB):
            nc.sync.dma_start(out=yt_sb[:, kb, :], in_=yt_d[kb])

        make_identity(nc, ident)
        nc.vector.memset(ones_sb, 1.0)

        # activations
        qt_sb = big.tile([128, r], bf)
        kt_sb = big.tile([128, r], bf)
        vt_sb = big.tile([128, r], bf)
        # v with a trailing ones column per (b, h, t-tile) block: [128, b, h, tt, 65]
        vaug_sb = big.tile([128, nbatch, HPC, ntt, 65], bf)
        attnT_sb = big.tile([64, HPC, r], bf)

        nc.vector.memset(vaug_sb[:, :, :, :, 64:65], 1.0)

        # ---------------- Phase P: projections ----------------
        with tc.tile_pool(name="projp", bufs=2, space="PSUM") as proj_pool:
            for wsb, bsb, dest in (
                (wq_sb, bq_sb, qt_sb),
                (wk_sb, bk_sb, kt_sb),
                (wv_sb, bv_sb, vt_sb),
            ):
                for o, w in nsplit(r, 2048):
                    ps = proj_pool.tile([128, 2048], f32, tag="proj")
                    for kb in range(KB):
                        for jo, jw in nsplit(w):
                            nc.tensor.matmul(
                                ps[:, jo : jo + jw],
                                lhsT=wsb[:, kb, :],
                                rhs=yt_sb[:, kb, o + jo : o + jo + jw],
                                start=(kb == 0),
                                stop=(kb == KB - 1),
                            )
                    nc.scalar.activation(
                        dest[:, o : o + w],
                        ps[:, :w],
                        AF.Identity,
                        bias=bsb,
                    )
            # transpose vT -> vaug (both heads at once per 128-column chunk)
            for ti in range(r // 128):
                b, tt = divmod(ti, ntt)
                tp = proj_pool.tile([128, 128], bf, tag="proj")
                nc.tensor.transpose(tp, vt_sb[:, ti * 128 : (ti + 1) * 128], ident)
                nc.vector.tensor_copy(
                    out=vaug_sb[:, b, :, tt, 0:64],
                    in_=tp.rearrange("p (h d) -> p h d", h=HPC),
                )

        # ---------------- Phase A: attention ----------------
        with (
            tc.tile_pool(name="scp", bufs=2, space="PSUM") as sc_pool,
            tc.tile_pool(name="atp", bufs=2, space="PSUM") as at_pool,
        ):
            for b in range(nbatch):
                for h in range(HPC):
                    hb = h * 64
                    for si in range(nsh):
                        s0 = b * seq + si * sh
                        acc = at_pool.tile([65, sh], f32, tag="attn")
                        for tt in range(ntt):
                            t0 = b * seq + tt * 128
                            sc = sc_pool.tile([128, sh], f32, tag="scores")
                            for jo, jw in nsplit(sh):
                                nc.tensor.matmul(
                                    sc[:, jo : jo + jw],
                                    lhsT=kt_sb[hb : hb + 64, t0 : t0 + 128],
                                    rhs=qt_sb[hb : hb + 64, s0 + jo : s0 + jo + jw],
                                    start=True,
                                    stop=True,
                                )
                            wt = wt_pool.tile([128, sh], bf, tag="wt")
                            nc.scalar.activation(wt, sc, AF.Exp, scale=float(1.0 / np.sqrt(D)))
                            for jo, jw in nsplit(sh):
                                nc.tensor.matmul(
                                    acc[:, jo : jo + jw],
                                    lhsT=vaug_sb[:, b, h, tt, :],
                                    rhs=wt[:, jo : jo + jw],
                                    start=(tt == 0),
                                    stop=(tt == ntt - 1),
                                    skip_group_check=True,
                                )
                        # normalize: attnT = acc[0:64] * (1 / acc[64]) broadcast
                        rec = rec_pool.tile([65, sh], bf, tag="rec")
                        with nc.allow_low_precision("softmax reciprocal in bf16"):
                            nc.vector.reciprocal(rec[64:65, :], acc[64:65, :])
                        rbc = sc_pool.tile([64, sh], f32, tag="scores")
                        for jo, jw in nsplit(sh):
                            nc.tensor.matmul(
                                rbc[:, jo : jo + jw],
                                lhsT=ones_sb[64:65, :],
                                rhs=rec[64:65, jo : jo + jw],
                                start=True,
                                stop=True,
                            )
                        # DVE can read only one PSUM operand per op: stage the
                        # broadcast reciprocal in SBUF, then multiply.
                        rbc_sb = rec_pool.tile([64, sh], f32, tag="rbc")
                        nc.vector.tensor_copy(out=rbc_sb, in_=rbc)
                        nc.vector.tensor_mul(
                            attnT_sb[:, h, s0 : s0 + sh], acc[0:64, :], rbc_sb
                        )

        # ---------------- Phase O: output projection ----------------
        with (
            tc.tile_pool(name="opp", bufs=2, space="PSUM") as op_pool,
            tc.tile_pool(name="ostg", bufs=3) as stg_pool,
        ):
            copy_idx = 0
            for mb in range(KB):
                for o, w in nsplit(r, 2048):
                    po = op_pool.tile([128, 2048], f32, tag="oproj")
                    for jo, jw in nsplit(w):
                        for h in range(HPC):
                            nc.tensor.matmul(
                                po[:, jo : jo + jw],
                                lhsT=wo_sb[:, h, mb, :],
                                rhs=attnT_sb[:, h, o + jo : o + jo + jw],
                                start=(h == 0),
                                stop=(h == HPC - 1),
                            )
                    stg = stg_pool.tile([128, 2048], f32, tag="ostg")
                    if copy_idx % 2 == 0:
                        nc.vector.tensor_copy(out=stg[:, :w], in_=po[:, :w])
                    else:
                        nc.scalar.copy(stg[:, :w], po[:, :w])
                    copy_idx += 1
                    nc.sync.dma_start(
                        out=out_d[mb * 128 : (mb + 1) * 128, o : o + w],
                        in_=stg[:, :w],
                    )

    nc.compile()
    return nc


def make_in_maps(x1, x2, Wq, bq, Wk, bk, Wv, bv, Wo, seq=S, nbatch=B):
    """Host-side sharding: build the 8 per-core input maps."""
    y = np.concatenate([np.asarray(x1), np.asarray(x2)], axis=1).astype(np.float32)
    r = nbatch * seq
    yT = np.ascontiguousarray(y.reshape(r, E).T)          # [E, r]
    yt_host = yT.reshape(KB, 128, r).astype(BF16)

    in_maps = []
    for c in range(NCORES):
        sl = slice(c * EPC, (c + 1) * EPC)

        def proj_w(W):
            # lhsT blocks: [e_in_128, kb, feat_128]
            A = np.ascontiguousarray(W[sl, :].T)          # [E, 128]
            return np.ascontiguousarray(
                A.reshape(KB, 128, 128).swapaxes(0, 1)
            ).astype(BF16)

        A = np.ascontiguousarray(np.asarray(Wo)[:, sl].T)  # [128(ei), E(eo)]
        wo_host = np.ascontiguousarray(
            A.reshape(HPC, 64, KB, 128).swapaxes(0, 1)
        ).astype(BF16)                                     # [64, h, mb, 128]

        in_maps.append(
            {
                "yt": yt_host,
                "wq": proj_w(np.asarray(Wq)),
                "wk": proj_w(np.asarray(Wk)),
                "wv": proj_w(np.asarray(Wv)),
                "wo": wo_host,
                "bq": np.ascontiguousarray(np.asarray(bq)[sl].reshape(128, 1)).astype(np.float32),
                "bk": np.ascontiguousarray(np.asarray(bk)[sl].reshape(128, 1)).astype(np.float32),
                "bv": np.ascontiguousarray(np.asarray(bv)[sl].reshape(128, 1)).astype(np.float32),
            }
        )
    return in_maps


_CACHE = {}


def _get_program():
    if "nc" not in _CACHE:
        _CACHE["nc"] = build_program()
    return _CACHE["nc"]


def kernel(x1, x2, Wq, bq, Wk, bk, Wv, bv, Wo, bo):
    from concourse.bass_utils import run_bass_kernel_spmd

    nc = _get_program()
    in_maps = make_in_maps(x1, x2, Wq, bq, Wk, bk, Wv, bv, Wo)
    res = run_bass_kernel_spmd(nc, in_maps, core_ids=list(range(NCORES)))
    _CACHE["last_results"] = res

    total = np.zeros((E, R), dtype=np.float32)
    for r in res.results:
        total += np.asarray(r["outp"], dtype=np.float32)
    out = total.T.reshape(B, S, E) + np.asarray(bo, dtype=np.float32)[None, None, :]
    return out.astype(np.float32)


# revision 7
# speedup vs baseline: 1.0191x; 1.0191x over previous
"""Cross-attention Trainium2 kernel, tensor-parallel over heads across 8 NeuronCores.

Problem (hardcoded): B=2, S1=S2=1024 (concat -> S=2048), E=1024, H=16, D=64.
Sharding: 2 heads per core (Megatron-style TP). Each core computes
  qT/kT/vT = column-sharded projections of y = concat(x1, x2)      [128, 4096]
  attnT    = softmax(q k^T / sqrt(D)) v   for its 2 heads          [64, 2, 4096]
  outT_c   = Wo[:, shard]^T-partial output projection              [1024, 4096]
Host sums the 8 partial outT's, transposes, adds bo.

All matmuls in bf16 (fp32 PSUM accumulate); softmax in fp32 on the ACT engine.
"""

import numpy as np
import ml_dtypes

BF16 = ml_dtypes.bfloat16

# Problem dims (hardcoded per contract)
B = 2
S = 2048          # concat sequence length per batch
E = 1024
H = 16
D = 64
NCORES = 8
HPC = H // NCORES  # heads per core = 2
EPC = HPC * D      # feature shard per core = 128
R = B * S          # flattened (b, s) columns = 4096
KB = E // 128      # contraction blocks for projections = 8


def build_program(seq=S, nbatch=B):
    """Build the SPMD Bass program (identical on all cores)."""
    from contextlib import ExitStack

    import concourse.bacc as bacc
    import concourse.tile as tile
    from concourse import mybir
    from concourse.masks import make_identity

    bf = mybir.dt.bfloat16
    f32 = mybir.dt.float32
    AF = mybir.ActivationFunctionType

    r = nbatch * seq            # total columns
    ntt = seq // 128            # t-tiles per batch
    sh = 1024 if seq % 1024 == 0 else seq   # s-half width
    nsh = seq // sh             # s-halves per batch
    assert seq % 128 == 0 and sh % 512 == 0 or sh < 512

    def nsplit(width, maxw=512):
        # split a free-dim width into <=512 chunks (PSUM bank limit per matmul)
        out = []
        o = 0
        while o < width:
            w = min(maxw, width - o)
            out.append((o, w))
            o += w
        return out

    nc = bacc.Bacc(
        "TRN2",
        target_bir_lowering=False,
        debug=False,
        enable_asserts=False,
        num_devices=NCORES,
    )

    # DRAM parameters (per-core data differs, program identical)
    yt_d = nc.dram_tensor("yt", [KB, 128, r], bf, kind="ExternalInput").ap()
    wq_d = nc.dram_tensor("wq", [128, KB, 128], bf, kind="ExternalInput").ap()
    wk_d = nc.dram_tensor("wk", [128, KB, 128], bf, kind="ExternalInput").ap()
    wv_d = nc.dram_tensor("wv", [128, KB, 128], bf, kind="ExternalInput").ap()
    wo_d = nc.dram_tensor("wo", [64, HPC, KB, 128], bf, kind="ExternalInput").ap()
    bq_d = nc.dram_tensor("bq", [128, 1], f32, kind="ExternalInput").ap()
    bk_d = nc.dram_tensor("bk", [128, 1], f32, kind="ExternalInput").ap()
    bv_d = nc.dram_tensor("bv", [128, 1], f32, kind="ExternalInput").ap()
    out_d = nc.dram_tensor("outp", [E, r], f32, kind="ExternalOutput").ap()

    with tile.TileContext(nc) as tc, ExitStack() as ctx:
        consts = ctx.enter_context(tc.tile_pool(name="consts", bufs=1))
        big = ctx.enter_context(tc.tile_pool(name="big", bufs=1))
        wt_pool = ctx.enter_context(tc.tile_pool(name="wtp", bufs=3))
        rec_pool = ctx.enter_context(tc.tile_pool(name="recp", bufs=2))

        # constants / weights
        wq_sb = consts.tile([128, KB, 128], bf)
        wk_sb = consts.tile([128, KB, 128], bf)
        wv_sb = consts.tile([128, KB, 128], bf)
        wo_sb = consts.tile([64, HPC, KB, 128], bf)
        bq_sb = consts.tile([128, 1], f32)
        bk_sb = consts.tile([128, 1], f32)
        bv_sb = consts.tile([128, 1], f32)
        ident = consts.tile([128, 128], bf)
        ones_sb = consts.tile([65, 64], bf)
        yt_sb = consts.tile([128, KB, r], bf)

        nc.sync.dma_start(out=wq_sb, in_=wq_d)
        nc.sync.dma_start(out=wk_sb, in_=wk_d)
        nc.sync.dma_start(out=wv_sb, in_=wv_d)
        nc.sync.dma_start(out=wo_sb, in_=wo_d)
        nc.sync.dma_start(out=bq_sb, in_=bq_d)
        nc.sync.dma_start(out=bk_sb, in_=bk_d)
        nc.sync.dma_start(out=bv_sb, in_=bv_d)
        for kb in range(KB):
            nc.sync.dma_start(out=yt_sb[:, kb, :], in_=yt_d[kb])

        make_identity(nc, ident)
        nc.vector.memset(ones_sb, 1.0)

        # activations
        qt_sb = big.tile([128, r], bf)
        kt_sb = big.tile([128, r], bf)
        vt_sb = big.tile([128, r], bf)
        # v with a trailing ones column per (b, h, t-tile) block: [128, b, h, tt, 65]
        vaug_sb = big.tile([128, nbatch, HPC, ntt, 65], bf)
        attnT_sb = big.tile([64, HPC, r], bf)

        nc.vector.memset(vaug_sb[:, :, :, :, 64:65], 1.0)

        # Single PSUM pool set for the whole kernel: no pool boundaries, so no
        # mid-kernel engine drains (they idle the PE >3.4us and trip the HAM
        # clock throttle back to 1.2 GHz).
        sc_pool = ctx.enter_context(tc.tile_pool(name="scp", bufs=2, space="PSUM"))
        at_pool = ctx.enter_context(tc.tile_pool(name="atp", bufs=2, space="PSUM"))
        stg_pool = ctx.enter_context(tc.tile_pool(name="ostg", bufs=3))

        # ---------------- Phase P: projections ----------------
        # v first: its transposes give the PE work while ACT copies q/k out of
        # PSUM, keeping the PE stream dense across the phase boundary.
        def project(wsb, bsb, dest, chunk=1024):
            for o, w in nsplit(r, chunk):
                ps = sc_pool.tile([128, 1024], f32, tag="scores")
                for kb in range(KB):
                    for jo, jw in nsplit(w):
                        nc.tensor.matmul(
                            ps[:, jo : jo + jw],
                            lhsT=wsb[:, kb, :],
                            rhs=yt_sb[:, kb, o + jo : o + jo + jw],
                            start=(kb == 0),
                            stop=(kb == KB - 1),
                        )
                nc.scalar.activation(
                    dest[:, o : o + w],
                    ps[:, :w],
                    AF.Identity,
                    bias=bsb,
                )

        project(wv_sb, bv_sb, vt_sb)
        project(wq_sb, bq_sb, qt_sb)
        # transpose vT -> vaug (both heads at once per 128-column chunk)
        for ti in range(r // 128):
            b, tt = divmod(ti, ntt)
            tp = at_pool.tile([128, 128], bf, tag="attn")
            nc.tensor.transpose(tp, vt_sb[:, ti * 128 : (ti + 1) * 128], ident)
            nc.vector.tensor_copy(
                out=vaug_sb[:, b, :, tt, 0:64],
                in_=tp.rearrange("p (h d) -> p h d", h=HPC),
            )
        project(wk_sb, bk_sb, kt_sb)

        # ---------------- Phase A: attention ----------------
        if True:
            for b in range(nbatch):
                for h in range(HPC):
                    hb = h * 64
                    for si in range(nsh):
                        s0 = b * seq + si * sh
                        acc = at_pool.tile([65, sh], f32, tag="attn")
                        for tt in range(ntt):
                            t0 = b * seq + tt * 128
                            sc = sc_pool.tile([128, sh], f32, tag="scores")
                            for jo, jw in nsplit(sh):
                                nc.tensor.matmul(
                                    sc[:, jo : jo + jw],
                                    lhsT=kt_sb[hb : hb + 64, t0 : t0 + 128],
                                    rhs=qt_sb[hb : hb + 64, s0 + jo : s0 + jo + jw],
                                    start=True,
                                    stop=True,
                                )
                            wt = wt_pool.tile([128, sh], bf, tag="wt")
                            nc.scalar.activation(wt, sc, AF.Exp, scale=float(1.0 / np.sqrt(D)))
                            for jo, jw in nsplit(sh):
                                nc.tensor.matmul(
                                    acc[:, jo : jo + jw],
                                    lhsT=vaug_sb[:, b, h, tt, :],
                                    rhs=wt[:, jo : jo + jw],
                                    start=(tt == 0),
                                    stop=(tt == ntt - 1),
                                    skip_group_check=True,
                                )
                        # normalize: attnT = acc[0:64] * (1 / acc[64]) broadcast
                        rec = rec_pool.tile([65, sh], bf, tag="rec")
                        with nc.allow_low_precision("softmax reciprocal in bf16"):
                            nc.vector.reciprocal(rec[64:65, :], acc[64:65, :])
                        rbc = sc_pool.tile([64, sh], f32, tag="scores")
                        for jo, jw in nsplit(sh):
                            nc.tensor.matmul(
                                rbc[:, jo : jo + jw],
                                lhsT=ones_sb[64:65, :],
                                rhs=rec[64:65, jo : jo + jw],
                                start=True,
                                stop=True,
                            )
                        # DVE can read only one PSUM operand per op: stage the
                        # broadcast reciprocal in SBUF, then multiply.
                        rbc_sb = rec_pool.tile([64, sh], f32, tag="rbc")
                        nc.vector.tensor_copy(out=rbc_sb, in_=rbc)
                        nc.vector.tensor_mul(
                            attnT_sb[:, h, s0 : s0 + sh], acc[0:64, :], rbc_sb
                        )

        # ---------------- Phase O: output projection ----------------
        copy_idx = 0
        for mb in range(KB):
            for o, w in nsplit(r, 1024):
                po = sc_pool.tile([128, 1024], f32, tag="scores")
                for jo, jw in nsplit(w):
                    for h in range(HPC):
                        nc.tensor.matmul(
                            po[:, jo : jo + jw],
                            lhsT=wo_sb[:, h, mb, :],
                            rhs=attnT_sb[:, h, o + jo : o + jo + jw],
                            start=(h == 0),
                            stop=(h == HPC - 1),
                        )
                stg = stg_pool.tile([128, 1024], f32, tag="ostg")
                if copy_idx % 2 == 0:
                    nc.vector.tensor_copy(out=stg[:, :w], in_=po[:, :w])
                else:
                    nc.scalar.copy(stg[:, :w], po[:, :w])
                copy_idx += 1
                nc.sync.dma_start(
                    out=out_d[mb * 128 : (mb + 1) * 128, o : o + w],
                    in_=stg[:, :w],
                )

    nc.compile()
    return nc


def make_in_maps(x1, x2, Wq, bq, Wk, bk, Wv, bv, Wo, seq=S, nbatch=B):
    """Host-side sharding: build the 8 per-core input maps."""
    y = np.concatenate([np.asarray(x1), np.asarray(x2)], axis=1).astype(np.float32)
    r = nbatch * seq
    yT = np.ascontiguousarray(y.reshape(r, E).T)          # [E, r]
    yt_host = yT.reshape(KB, 128, r).astype(BF16)

    in_maps = []
    for c in range(NCORES):
        sl = slice(c * EPC, (c + 1) * EPC)

        def proj_w(W):
            # lhsT blocks: [e_in_128, kb, feat_128]
            A = np.ascontiguousarray(W[sl, :].T)          # [E, 128]
            return np.ascontiguousarray(
                A.reshape(KB, 128, 128).swapaxes(0, 1)
            ).astype(BF16)

        A = np.ascontiguousarray(np.asarray(Wo)[:, sl].T)  # [128(ei), E(eo)]
        wo_host = np.ascontiguousarray(
            A.reshape(HPC, 64, KB, 128).swapaxes(0, 1)
        ).astype(BF16)                                     # [64, h, mb, 128]

        in_maps.append(
            {
                "yt": yt_host,
                "wq": proj_w(np.asarray(Wq)),
                "wk": proj_w(np.asarray(Wk)),
                "wv": proj_w(np.asarray(Wv)),
                "wo": wo_host,
                "bq": np.ascontiguousarray(np.asarray(bq)[sl].reshape(128, 1)).astype(np.float32),
                "bk": np.ascontiguousarray(np.asarray(bk)[sl].reshape(128, 1)).astype(np.float32),
                "bv": np.ascontiguousarray(np.asarray(bv)[sl].reshape(128, 1)).astype(np.float32),
            }
        )
    return in_maps


_CACHE = {}


def _get_program():
    if "nc" not in _CACHE:
        _CACHE["nc"] = build_program()
    return _CACHE["nc"]


def kernel(x1, x2, Wq, bq, Wk, bk, Wv, bv, Wo, bo):
    from concourse.bass_utils import run_bass_kernel_spmd

    nc = _get_program()
    in_maps = make_in_maps(x1, x2, Wq, bq, Wk, bk, Wv, bv, Wo)
    res = run_bass_kernel_spmd(nc, in_maps, core_ids=list(range(NCORES)))
    _CACHE["last_results"] = res

    total = np.zeros((E, R), dtype=np.float32)
    for r in res.results:
        total += np.asarray(r["outp"], dtype=np.float32)
    out = total.T.reshape(B, S, E) + np.asarray(bo, dtype=np.float32)[None, None, :]
    return out.astype(np.float32)
